# revision 1
# baseline (speedup 1.0000x reference)
"""Fused LayerNorm + multi-head attention + output projection for TRN2.

Sharding over 8 NeuronCores: core c handles batch c//2 and head-half c%2
(8 of 16 heads). Head-parallel QKV/attention, row-parallel proj; the
cross-core reduction of proj partials happens on the host during unshard
(pairs of cores share a batch).

Device layout notes:
  - LayerNorm gamma/beta and the attention scale are folded into w_qkv /
    b_qkv on the host, so the device only computes (x - mu) * rstd.
  - q,k are produced transposed ([cols, tokens]) so Q.K^T needs no extra
    transposes; v is produced token-major with an appended ones column so
    the P.V matmul also yields the softmax denominator (row 64 of PSUM).
  - exp() runs without max-subtraction: logits are ~N(0,1) here, fp32 exp
    is exact enough and cannot overflow.
"""

import sys

sys.path.insert(0, "/opt/trn_rl_repo")

import numpy as np
import ml_dtypes

N = 2048          # tokens per batch
D = 1024          # model dim
HL = 8            # heads per core
DH = 64           # head dim
INNER_L = HL * DH  # 512, per-core inner width
TT = N // 128     # 16 token tiles
KC = D // 128     # 8 dim chunks
SCALE = DH ** -0.5

BF16 = ml_dtypes.bfloat16

_CACHE = {}


def _build_nc():
    import concourse.bass as bass
    import concourse.mybir as mybir
    import concourse.tile as tile
    from concourse import bacc

    F32 = mybir.dt.float32
    F32R = mybir.dt.float32r
    BF = mybir.dt.bfloat16
    AF = mybir.ActivationFunctionType
    OP = mybir.AluOpType

    nc = bacc.Bacc("TRN2", target_bir_lowering=False)

    x_in = nc.declare_dram_parameter("x", [N, D], F32, isOutput=False)
    wqkv_in = nc.declare_dram_parameter("wqkv", [D, 2 * INNER_L + INNER_L], BF, isOutput=False)
    bqk_in = nc.declare_dram_parameter("bqk", [128, 8], F32, isOutput=False)
    bv_in = nc.declare_dram_parameter("bv", [128, INNER_L], F32, isOutput=False)
    wproj_in = nc.declare_dram_parameter("wproj", [INNER_L, D], BF, isOutput=False)
    ident_in = nc.declare_dram_parameter("ident", [128, 128], F32, isOutput=False)
    out_ext = nc.declare_dram_parameter("out", [N, D], F32, isOutput=True)


    with tile.TileContext(nc) as tc:
        with (
            tc.tile_pool(name="persist", bufs=1) as persist,
            tc.tile_pool(name="xload", bufs=3) as xload,
            tc.tile_pool(name="lnstat", bufs=6) as lnstat,
            tc.tile_pool(name="ptile", bufs=4) as ptile,
            tc.tile_pool(name="lrow", bufs=2) as lrow,
            tc.tile_pool(name="outsb", bufs=2) as outsb,
            tc.tile_pool(name="ldram", bufs=3, space="DRAM") as ldram,
            tc.tile_pool(name="ps3", bufs=2, space="PSUM") as ps3,
            tc.tile_pool(name="ps_o", bufs=2, space="PSUM") as ps_o,
        ):
            # ---- persistent tiles ----
            w_sb = persist.tile([128, KC, 1536], BF, tag="w_sb")
            wproj_sb = persist.tile([128, 4, D], BF, tag="wproj_sb")
            bqk_sb = persist.tile([128, 8], F32, tag="bqk_sb")
            bv_sb = persist.tile([128, INNER_L], F32, tag="bv_sb")
            ident = persist.tile([128, 128], F32, tag="ident")
            eps_t = persist.tile([128, 1], F32, tag="eps_t")
            xnT = persist.tile([128, KC, N], BF, tag="xnT")
            qkT = persist.tile([128, 8, N], BF, tag="qkT")
            v_all = persist.tile([128, TT, HL, DH + 1], BF, tag="v_all")
            ocatT = persist.tile([128, 4, N], BF, tag="ocatT")

            # ident first: the first PE transposes gate on it. Route it
            # through a DVE copy so the transpose needs only one wait proc
            # (PE instructions have a tight HW wait-slot budget).
            ident_raw = persist.tile([128, 128], F32, tag="ident_raw")
            nc.sync.dma_start(out=ident_raw, in_=ident_in[:, :])
            nc.vector.tensor_copy(out=ident, in_=ident_raw)
            nc.sync.dma_start(out=bqk_sb, in_=bqk_in[:, :])
            nc.sync.dma_start(out=bv_sb, in_=bv_in[:, :])
            # v columns first so the phase-1 v matmuls unblock early
            nc.sync.dma_start(
                out=w_sb[:, :, 1024:1536],
                in_=wqkv_in[:, 1024:1536].rearrange("(c p) d -> p c d", p=128),
            )
            nc.sync.dma_start(
                out=w_sb[:, :, 0:1024],
                in_=wqkv_in[:, 0:1024].rearrange("(c p) d -> p c d", p=128),
            )
            nc.sync.dma_start(
                out=wproj_sb, in_=wproj_in[:, :].rearrange("(c p) d -> p c d", p=128)
            )
            nc.vector.memset(eps_t, 1e-5)
            # ones column at [..., 64]: set the whole tile to 1.0 (contiguous
            # memset; strided 4-D memset fails ISA checks), the per-head
            # tensor_add writes then overwrite cols 0..63 of each head.
            nc.vector.memset(v_all, 1.0)

            # ---- phase 1: LayerNorm + transpose into xnT ----
            # x loads batched 4 token-tiles per DMA (amortize DMA fixed cost)
            for tq in range(TT // 2):
                xb = xload.tile([128, 2, D], F32, tag="xb")
                xdma = nc.gpsimd if tq % 2 == 0 else nc.sync
                xdma.dma_start(
                    out=xb,
                    in_=x_in[tq * 256:(tq + 1) * 256, :].rearrange("(c p) d -> p c d", p=128),
                )
                for c in range(2):
                    t = tq * 2 + c
                    xt = xb[:, c, :]
                    stats = lnstat.tile([128, 2, 6], F32, tag="stats")
                    nc.vector.bn_stats(out=stats[:, 0, :], in_=xt[:, 0:512])
                    nc.vector.bn_stats(out=stats[:, 1, :], in_=xt[:, 512:1024])
                    mv = lnstat.tile([128, 2], F32, tag="mv")
                    nc.vector.bn_aggr(out=mv, in_=stats)
                    # mv[:,0]=mean, mv[:,1]=var -> std -> rstd
                    nc.scalar.activation(out=mv[:, 1:2], in_=mv[:, 1:2], func=AF.Sqrt, bias=eps_t)
                    rstd = lnstat.tile([128, 1], F32, tag="rstd")
                    nc.vector.reciprocal(out=rstd, in_=mv[:, 1:2])
                    nc.vector.tensor_scalar(
                        out=xt, in0=xt, scalar1=mv[:, 0:1], scalar2=rstd,
                        op0=OP.subtract, op1=OP.mult,
                    )
                    ptr = ps3.tile([128, D], F32, tag="ps3")
                    for kc in range(KC):
                        nc.tensor.transpose(
                            out=ptr[:, kc * 128:(kc + 1) * 128],
                            in_=xt[:, kc * 128:(kc + 1) * 128],
                            identity=ident,
                        )
                    nc.vector.tensor_copy(
                        out=xnT[:, :, t * 128:(t + 1) * 128], in_=ptr.rearrange("p (k t) -> p k t", k=KC)
                    )
                    # v matmul for this tile right away (only needs its own
                    # xnT slice) — keeps the PE fed during the DMA/LN-paced
                    # startup phase
                    pv = ps3.tile([128, 512], F32, tag="ps3")
                    for kc in range(KC):
                        nc.tensor.matmul(
                            out=pv,
                            lhsT=xnT[:, kc, t * 128:(t + 1) * 128],
                            rhs=w_sb[:, kc, 1024:1536],
                            start=(kc == 0), stop=(kc == KC - 1),
                        )
                    nc.vector.tensor_add(
                        out=v_all[:, t, :, 0:DH],
                        in0=pv.rearrange("p (h d) -> p h d", h=HL),
                        in1=bv_sb.rearrange("p (h d) -> p h d", h=HL),
                    )

            # ---- phases 2b+3 interleaved per head pair: the dense qk
            # accumulation runs keep the PE HAM clock-gate warm between the
            # attention chains ----
            def emit_qk(mt):
                for half in range(2):
                    pqk = ps3.tile([128, 1024], F32, tag="ps3")
                    for ns in range(2):
                        tok0 = half * 1024 + ns * 512
                        for kc in range(KC):
                            nc.tensor.matmul(
                                out=pqk[:, ns * 512:(ns + 1) * 512],
                                lhsT=w_sb[:, kc, mt * 128:(mt + 1) * 128],
                                rhs=xnT[:, kc, tok0:tok0 + 512],
                                start=(kc == 0), stop=(kc == KC - 1),
                            )
                    nc.vector.tensor_scalar(
                        out=qkT[:, mt, half * 1024:(half + 1) * 1024],
                        in0=pqk, scalar1=bqk_sb[:, mt:mt + 1], scalar2=None,
                        op0=OP.add,
                    )

            def emit_proj(tq):
                ob = outsb.tile([128, 2, D], F32, tag="ob")
                for c in range(2):
                    t = tq * 2 + c
                    pp = ps3.tile([128, 1024], F32, tag="ps3")
                    for ns in range(2):
                        for kc in range(4):
                            nc.tensor.matmul(
                                out=pp[:, ns * 512:(ns + 1) * 512],
                                lhsT=ocatT[:, kc, t * 128:(t + 1) * 128],
                                rhs=wproj_sb[:, kc, ns * 512:(ns + 1) * 512],
                                start=(kc == 0), stop=(kc == 3),
                            )
                    nc.vector.tensor_copy(out=ob[:, c, :], in_=pp)
                nc.sync.dma_start(
                    out=out_ext[tq * 256:(tq + 1) * 256, :].rearrange("(c p) d -> p c d", p=128),
                    in_=ob,
                )

            for h in range(HL):
                if h % 2 == 0:
                    emit_qk(h // 2)       # q tile for this head pair
                    emit_qk(4 + h // 2)   # k tile for this head pair
                for ihalf in range(2):
                    hq = h // 2
                    hp = (h % 2) * 64
                    po = ps_o.tile([128, 1024], F32, tag="po")
                    for jc in range(TT):
                        pst = ps3.tile([128, 1024], F32, tag="ps3")
                        for ns in range(2):
                            i0 = ihalf * 1024 + ns * 512
                            nc.tensor.matmul(
                                out=pst[:, ns * 512:(ns + 1) * 512],
                                lhsT=qkT[hp:hp + 64, 4 + hq, jc * 128:(jc + 1) * 128],
                                rhs=qkT[hp:hp + 64, hq, i0:i0 + 512],
                                start=True, stop=True,
                            )
                        pT = ptile.tile([128, 1024], BF, tag="pT")
                        nc.scalar.activation(out=pT, in_=pst, func=AF.Exp)
                        for ns in range(2):
                            nc.tensor.matmul(
                                out=po[0:65, ns * 512:(ns + 1) * 512],
                                lhsT=v_all[:, jc, h, :],
                                rhs=pT[:, ns * 512:(ns + 1) * 512],
                                start=(jc == 0), stop=(jc == TT - 1),
                            )
                    # denominator: row 64 of po; reciprocal then broadcast to
                    # partitions 64..128 via a K=1 matmul, then O / l on DVE.
                    # Normalize O by the softmax denominator l (row 64 of po):
                    # fast-reciprocal the l row, replicate it to 64 partitions
                    # with a partition-stride-0 DMA (keeps the in-order PE
                    # queue free of normalization work), then multiply.
                    lrow_s = lrow.tile([1, 1024], F32, tag="lrow_s")
                    nc.vector.tensor_copy(out=lrow_s, in_=po[64:65, :])
                    linv = lrow.tile([1, 1024], F32, tag="linv")
                    nc.vector.reciprocal_approx_fast(out=linv, in_=lrow_s)
                    linb = lrow.tile([64, 1024], F32, tag="linb")
                    lb = ldram.tile([1, 1024], F32, tag="lb")
                    nc.sync.dma_start(out=lb, in_=linv)
                    lb_bc = bass.AP(
                        tensor=lb.tensor, offset=lb.offset,
                        ap=[[0, 64]] + lb.ap[1:],
                    )
                    nc.sync.dma_start(out=linb, in_=lb_bc)
                    nc.vector.tensor_mul(
                        out=ocatT[hp:hp + 64, hq, ihalf * 1024:(ihalf + 1) * 1024],
                        in0=po[0:64, :], in1=linb,
                    )
            for tq in range(TT // 2):
                emit_proj(tq)

    # Bacc defers register allocation etc. to compile(), which runs via
    # finalize(); the axon/pjrt exec path serializes the BIR as-is, so
    # finalize here.
    nc.finalize()
    return nc


def _prep_in_maps(x, ln_gamma, ln_beta, w_qkv, b_qkv, w_proj):
    x = np.asarray(x, dtype=np.float32)
    ln_gamma = np.asarray(ln_gamma, dtype=np.float32)
    ln_beta = np.asarray(ln_beta, dtype=np.float32)
    w_qkv = np.asarray(w_qkv, dtype=np.float32)
    b_qkv = np.asarray(b_qkv, dtype=np.float32)
    w_proj = np.asarray(w_proj, dtype=np.float32)

    W = ln_gamma[:, None] * w_qkv          # fold gamma
    beff = b_qkv + ln_beta @ w_qkv         # fold beta
    ident = np.eye(128, dtype=np.float32)

    in_maps = []
    for c in range(8):
        b, half = divmod(c, 2)
        hs = half * INNER_L
        wq = W[:, hs:hs + INNER_L] * SCALE
        wk = W[:, D + hs:D + hs + INNER_L]
        wv = W[:, 2 * D + hs:2 * D + hs + INNER_L]
        bq = beff[hs:hs + INNER_L] * SCALE
        bk = beff[D + hs:D + hs + INNER_L]
        bv = beff[2 * D + hs:2 * D + hs + INNER_L]
        wqkv_c = np.ascontiguousarray(
            np.concatenate([wq, wk, wv], axis=1)
        ).astype(BF16)
        bqk_col = np.ascontiguousarray(
            np.concatenate([bq, bk]).reshape(8, 128).T
        )
        bv_bc = np.ascontiguousarray(np.broadcast_to(bv[None, :], (128, INNER_L)))
        wproj_c = np.ascontiguousarray(w_proj[hs:hs + INNER_L, :]).astype(BF16)
        in_maps.append({
            "x": np.ascontiguousarray(x[b]),
            "wqkv": wqkv_c,
            "bqk": bqk_col,
            "bv": bv_bc,
            "wproj": wproj_c,
            "ident": ident,
        })
    return in_maps


def kernel(x, ln_gamma, ln_beta, w_qkv, b_qkv, w_proj, b_proj, _trace=False, _tmpdir=None):
    from concourse.bass_utils import run_bass_kernel_spmd

    if "nc" not in _CACHE:
        _CACHE["nc"] = _build_nc()
    nc = _CACHE["nc"]

    in_maps = _prep_in_maps(x, ln_gamma, ln_beta, w_qkv, b_qkv, w_proj)
    res = run_bass_kernel_spmd(
        nc, in_maps, core_ids=list(range(8)), trace=_trace, tmpdir=_tmpdir
    )
    _CACHE["last_result"] = res

    b_proj = np.asarray(b_proj, dtype=np.float32)
    out = np.empty((4, N, D), dtype=np.float32)
    for b in range(4):
        out[b] = res.results[2 * b]["out"] + res.results[2 * b + 1]["out"] + b_proj
    return out



# revision 7
# speedup vs baseline: 1.1140x; 1.1140x over previous
"""Fused LayerNorm + multi-head attention + output projection for TRN2.

Sharding over 8 NeuronCores: core c handles batch c//2 and head-half c%2
(8 of 16 heads). Head-parallel QKV/attention, row-parallel proj; the
cross-core reduction of proj partials happens on the host during unshard
(pairs of cores share a batch).

Schedule notes (v2):
  - The softmax exp stream on the ACT engine (256 x ~1.15us) and the PE
    matmul stream are co-critical (~290us each). The kernel keeps ACT
    saturated by software-pipelining scores(jc+1) ahead of exp(jc)/PV(jc)
    and feeding the PE's slack with qk-gen / proj "filler" matmuls.
  - Scores matmuls have K=64 (head dim), so two heads' scores run
    CONCURRENTLY on disjoint PE row-groups (tile_position row tiling):
    head A lives at partitions 0-63 of qkT, head B at 64-127, which is
    exactly the layout the qk-gen matmuls produce.
  - x arrives bf16 (halves DMA + 2x DVE LayerNorm); transposes run in
    bf16 (1 cycle/row vs fp32's ~4).
  - v is stored [128, 65, tile, head] so the softmax-denominator ones
    column is one contiguous memset, and the PV matmul for (tile, head)
    reads a strided [128, 65] lhsT.
  - exp() runs without max-subtraction: logits are ~N(0,1), fp32 exp
    cannot overflow and bf16 P is accurate enough.
"""

import sys

sys.path.insert(0, "/opt/trn_rl_repo")

import numpy as np
import ml_dtypes

N = 2048          # tokens per batch
D = 1024          # model dim
HL = 8            # heads per core
DH = 64           # head dim
INNER_L = HL * DH  # 512, per-core inner width
TT = N // 128     # 16 token tiles
KC = D // 128     # 8 dim chunks
NQ = 4            # query quarters per unit set
QW = N // NQ      # 512 queries per quarter
SCALE = DH ** -0.5

BF16 = ml_dtypes.bfloat16

_CACHE = {}


def _build_nc():
    import concourse.bass as bass
    import concourse.mybir as mybir
    import concourse.tile as tile
    from concourse import bacc

    F32 = mybir.dt.float32
    BF = mybir.dt.bfloat16
    AF = mybir.ActivationFunctionType
    OP = mybir.AluOpType

    nc = bacc.Bacc("TRN2", target_bir_lowering=False)

    x_in = nc.declare_dram_parameter("x", [N, D], BF, isOutput=False)
    wqkv_in = nc.declare_dram_parameter("wqkv", [D, 3 * INNER_L], BF, isOutput=False)
    bqk_in = nc.declare_dram_parameter("bqk", [128, 8], F32, isOutput=False)
    bv_in = nc.declare_dram_parameter("bv", [128, INNER_L], F32, isOutput=False)
    wproj_in = nc.declare_dram_parameter("wproj", [INNER_L, D], BF, isOutput=False)
    ident_in = nc.declare_dram_parameter("ident", [128, 128], BF, isOutput=False)
    out_ext = nc.declare_dram_parameter("out", [N, D], BF, isOutput=True)

    with tile.TileContext(nc) as tc:
        with (
            tc.tile_pool(name="persist", bufs=1) as persist,
            tc.tile_pool(name="xload", bufs=3) as xload,
            tc.tile_pool(name="lnstat", bufs=6) as lnstat,
            tc.tile_pool(name="ptile", bufs=3) as ptile,
            tc.tile_pool(name="oraw", bufs=2) as orawp,
            tc.tile_pool(name="lrow", bufs=2) as lrow,
            tc.tile_pool(name="outsb", bufs=2) as outsb,
            tc.tile_pool(name="ldram", bufs=2, space="DRAM") as ldram,
            tc.tile_pool(name="pst", bufs=2, space="PSUM") as pstp,
            tc.tile_pool(name="po", bufs=1, space="PSUM") as pop,
            tc.tile_pool(name="fill", bufs=2, space="PSUM") as fillp,
        ):
            # ---- persistent tiles ----
            w_sb = persist.tile([128, KC, 1536], BF, tag="w_sb")
            wproj_sb = persist.tile([128, 4, D], BF, tag="wproj_sb")
            bqk_sb = persist.tile([128, 8], F32, tag="bqk_sb")
            bv_sb = persist.tile([128, INNER_L], F32, tag="bv_sb")
            ident = persist.tile([128, 128], BF, tag="ident")
            eps_t = persist.tile([128, 1], F32, tag="eps_t")
            xnT = persist.tile([128, KC, N], BF, tag="xnT")
            qkT = persist.tile([128, 8, N], BF, tag="qkT")
            v_all = persist.tile([128, DH + 1, TT, HL], BF, tag="v_all")
            ocatT = persist.tile([128, 4, N], BF, tag="ocatT")

            # ident routed through a DVE copy so the transposes need only
            # one wait proc (PE instructions have a tight wait-slot budget).
            ident_raw = persist.tile([128, 128], BF, tag="ident_raw")
            nc.sync.dma_start(out=ident_raw, in_=ident_in[:, :])
            nc.vector.tensor_copy(out=ident, in_=ident_raw)
            nc.sync.dma_start(out=bqk_sb, in_=bqk_in[:, :])
            nc.sync.dma_start(out=bv_sb, in_=bv_in[:, :])
            nc.vector.memset(eps_t, 1e-5)
            # softmax-denominator ones column: contiguous [128, 1, 16, 8]
            nc.vector.memset(v_all[:, DH:DH + 1, :, :], 1.0)

            # weight DMAs, ordered by first use: v cols (phase-1 v matmuls),
            # then q/k tiles for head pair 0, then the rest, proj last.
            def dma_w(col0, col1):
                nc.sync.dma_start(
                    out=w_sb[:, :, col0:col1],
                    in_=wqkv_in[:, col0:col1].rearrange("(c p) d -> p c d", p=128),
                )

            dma_w(1024, 1536)          # v
            dma_w(0, 128)              # q mt0
            dma_w(512, 640)            # k mt4
            for mt in (1, 5, 2, 6, 3, 7):
                dma_w(mt * 128, (mt + 1) * 128)
            nc.sync.dma_start(
                out=wproj_sb, in_=wproj_in[:, :].rearrange("(c p) d -> p c d", p=128)
            )

            # ---- phase 1 per-tile: load + LayerNorm + transpose + v ----
            def emit_tile(t):
                xb = xload.tile([128, D], BF, tag="xb")
                nc.gpsimd.dma_start(
                    out=xb,
                    in_=x_in[t * 128:(t + 1) * 128, :],
                )
                stats = lnstat.tile([128, 2, 6], F32, tag="stats")
                nc.vector.bn_stats(out=stats[:, 0, :], in_=xb[:, 0:512])
                nc.vector.bn_stats(out=stats[:, 1, :], in_=xb[:, 512:1024])
                mv = lnstat.tile([128, 2], F32, tag="mv")
                nc.vector.bn_aggr(out=mv, in_=stats)
                nc.scalar.activation(out=mv[:, 1:2], in_=mv[:, 1:2], func=AF.Sqrt, bias=eps_t)
                rstd = lnstat.tile([128, 1], F32, tag="rstd")
                nc.vector.reciprocal(out=rstd, in_=mv[:, 1:2])
                nc.vector.tensor_scalar(
                    out=xb, in0=xb, scalar1=mv[:, 0:1], scalar2=rstd,
                    op0=OP.subtract, op1=OP.mult,
                )
                ptr = fillp.tile([128, D], BF, tag="fill")
                for kc in range(KC):
                    nc.tensor.transpose(
                        out=ptr[:, kc * 128:(kc + 1) * 128],
                        in_=xb[:, kc * 128:(kc + 1) * 128],
                        identity=ident,
                    )
                nc.vector.tensor_copy(
                    out=xnT[:, :, t * 128:(t + 1) * 128],
                    in_=ptr.rearrange("p (k t) -> p k t", k=KC),
                )
                pv = fillp.tile([128, 512], F32, tag="fill")
                for kc in range(KC):
                    nc.tensor.matmul(
                        out=pv,
                        lhsT=xnT[:, kc, t * 128:(t + 1) * 128],
                        rhs=w_sb[:, kc, 1024:1536],
                        start=(kc == 0), stop=(kc == KC - 1),
                    )
                nc.vector.tensor_add(
                    out=v_all[:, 0:DH, t, :],
                    in0=pv.rearrange("p (h d) -> p d h", h=HL),
                    in1=bv_sb.rearrange("p (h d) -> p d h", h=HL),
                )

            # qk-gen: one (mt, chunk) produces qkT[:, mt, ch*512:(ch+1)*512].
            # Split into two 4-matmul quanta so it can interleave with the
            # attention stream without starving the ACT engine.
            def qk_quantum(mt, ch, half, pqk):
                for kc in range(4 * half, 4 * half + 4):
                    nc.tensor.matmul(
                        out=pqk,
                        lhsT=w_sb[:, kc, mt * 128:(mt + 1) * 128],
                        rhs=xnT[:, kc, ch * 512:(ch + 1) * 512],
                        start=(kc == 0), stop=(kc == KC - 1),
                    )
                if half == 1:
                    nc.vector.tensor_scalar(
                        out=qkT[:, mt, ch * 512:(ch + 1) * 512],
                        in0=pqk, scalar1=bqk_sb[:, mt:mt + 1], scalar2=None,
                        op0=OP.add,
                    )

            def make_qk_actions(mt):
                acts = []
                for ch in range(4):
                    state = {}

                    def a0(mt=mt, ch=ch, state=state):
                        state["pqk"] = fillp.tile([128, 512], F32, tag="fill", name="pqk_f")
                        qk_quantum(mt, ch, 0, state["pqk"])

                    def a1(mt=mt, ch=ch, state=state):
                        qk_quantum(mt, ch, 1, state["pqk"])

                    acts += [a0, a1]
                return acts

            # proj for one token tile: one quantum per 512-wide ns chunk
            # (keeps every fill-pool tile at one PSUM bank). bf16 partials;
            # host adds the pair of cores + b_proj.
            def make_proj_actions(q):
                acts = []
                for tp in range(2):  # two token-tile pairs per quarter
                    state = {"q": q, "t0": q * 4 + tp * 2}

                    def quantum(c, ns, state=state):
                        def a():
                            t = state["t0"] + c
                            if c == 0 and ns == 0:
                                state["ob"] = outsb.tile([128, 2, D], BF, tag="ob", name="ob")
                            pp = fillp.tile([128, 512], F32, tag="fill")
                            for kc in range(4):
                                nc.tensor.matmul(
                                    out=pp,
                                    lhsT=ocatT[:, kc, t * 128:(t + 1) * 128],
                                    rhs=wproj_sb[:, kc, ns * 512:(ns + 1) * 512],
                                    start=(kc == 0), stop=(kc == 3),
                                )
                            nc.vector.tensor_copy(
                                out=state["ob"][:, c, ns * 512:(ns + 1) * 512], in_=pp)
                            if c == 1 and ns == 1:
                                t0 = state["t0"]
                                nc.sync.dma_start(
                                    out=out_ext[t0 * 128:(t0 + 2) * 128, :].rearrange(
                                        "(c p) d -> p c d", p=128),
                                    in_=state["ob"],
                                )
                        return a

                    for c in range(2):
                        for ns in range(2):
                            acts.append(quantum(c, ns))
                return acts

            # ---- emit phase 1 (tiles + early qk for head pair 0) ----
            for t in range(TT):
                emit_tile(t)
                if t % 4 == 3:
                    ch = t // 4
                    for mt in (0, 4):
                        pqk = fillp.tile([128, 512], F32, tag="fill")
                        qk_quantum(mt, ch, 0, pqk)
                        qk_quantum(mt, ch, 1, pqk)

            # ---- attention: globally software-pipelined stream ----
            # units: (hpair, quarter), hpair-outer. Per unit 16 jc steps:
            #   S_pair(jc): two row-tiled matmuls -> pst [128, 0:512]=A,
            #               [512:1024]=B
            #   E(jc): one exp [128, 1024] -> pT bf16
            #   P(jc): two PV matmuls -> po[0:65, 0:512]=A, [512:1024]=B
            units = [(hp, q) for hp in range(4) for q in range(NQ)]

            # filler actions allowed per unit index
            filler = {u: [] for u in range(len(units))}
            qk_acts = {h: make_qk_actions(h) + make_qk_actions(4 + h) for h in (1, 2, 3)}
            for h in (1, 2, 3):
                acts = qk_acts[h]
                # spread over the 4 units of hpair h-1
                for i, a in enumerate(acts):
                    filler[(h - 1) * 4 + i * 4 // len(acts)].append(a)
            for q in range(NQ - 1):
                # proj(q) during unit (hpair3, q+1)
                for a in make_proj_actions(q):
                    filler[12 + q + 1].append(a)

            state = {"pst": None, "pT": None, "po": None, "prev": None}

            def emit_S(u, jc):
                hp, q = units[u]
                pst = pstp.tile([128, 1024], F32, tag="pst")
                for half in range(2):
                    p0 = half * 64
                    nc.tensor.matmul(
                        out=pst[:, half * 512:(half + 1) * 512],
                        lhsT=qkT[p0:p0 + 64, 4 + hp, jc * 128:(jc + 1) * 128],
                        rhs=qkT[p0:p0 + 64, hp, q * 512:(q + 1) * 512],
                        start=True, stop=True,
                    )
                return pst

            def emit_EP(u, jc, pst, po):
                hp, q = units[u]
                pT = ptile.tile([128, 1024], BF, tag="pT")
                nc.scalar.activation(out=pT, in_=pst, func=AF.Exp)
                for half in range(2):
                    h = 2 * hp + half
                    nc.tensor.matmul(
                        out=po[0:DH + 1, half * 512:(half + 1) * 512],
                        lhsT=v_all[:, :, jc, h],
                        rhs=pT[:, half * 512:(half + 1) * 512],
                        start=(jc == 0), stop=(jc == TT - 1),
                    )

            def emit_release(u, po):
                hp, q = units[u]
                oraw = orawp.tile([DH + 1, 1024], F32, tag="oraw")
                nc.vector.tensor_copy(out=oraw, in_=po[0:DH + 1, :])
                # reciprocal_approx_fast (custom DVE op) requires its input at
                # partition 0 — with a partition-64 input it returns garbage on
                # HW (sim does not model this). Stage the l row through a
                # partition-0 tile first; plain tensor_copy handles the
                # partition crossing fine.
                lrow_s = lrow.tile([1, 1024], F32, tag="lrow_s")
                nc.vector.tensor_copy(out=lrow_s, in_=oraw[DH:DH + 1, :])
                linv = lrow.tile([1, 1024], F32, tag="linv")
                nc.vector.reciprocal_approx_fast(out=linv, in_=lrow_s)
                lb = ldram.tile([1, 1024], F32, tag="lb")
                nc.sync.dma_start(out=lb, in_=linv)
                linb = lrow.tile([64, 1024], F32, tag="linb")
                lb_bc = bass.AP(
                    tensor=lb.tensor, offset=lb.offset,
                    ap=[[0, 64]] + lb.ap[1:],
                )
                nc.sync.dma_start(out=linb, in_=lb_bc)
                for half in range(2):
                    nc.vector.tensor_mul(
                        out=ocatT[half * 64:(half + 1) * 64, hp,
                                  q * 512:(q + 1) * 512],
                        in0=oraw[0:64, half * 512:(half + 1) * 512],
                        in1=linb[:, half * 512:(half + 1) * 512],
                    )

            for u in range(len(units)):
                po = pop.tile([128, 1024], F32, tag="po")
                fq = list(filler[u])
                fi = 0
                # proj filler needs this-unit's predecessor release emitted
                # first (it reads ocatT written at jc==0 of this unit), so
                # those units gate pops to jc >= 4 and pop every step.
                is_proj = u >= 13
                for jc in range(TT):
                    pst = emit_S(u, jc)
                    prev = state["prev"]
                    if prev is not None:
                        emit_EP(prev[0], prev[1], state["pst"], state["po"])
                        if prev[1] == TT - 1:
                            emit_release(prev[0], state["po"])
                    if fi < len(fq) and (jc >= 4 if is_proj else jc % 2 == 0):
                        fq[fi]()
                        fi += 1
                    state["pst"], state["po"], state["prev"] = pst, po, (u, jc)
                # drain any leftover filler at unit end
                while fi < len(fq):
                    fq[fi]()
                    fi += 1
            # drain the last pipelined step
            prev = state["prev"]
            emit_EP(prev[0], prev[1], state["pst"], state["po"])
            emit_release(prev[0], state["po"])
            # final quarter's proj
            for a in make_proj_actions(NQ - 1):
                a()

    nc.finalize()
    return nc


def _prep_in_maps(x, ln_gamma, ln_beta, w_qkv, b_qkv, w_proj):
    x = np.asarray(x, dtype=np.float32)
    ln_gamma = np.asarray(ln_gamma, dtype=np.float32)
    ln_beta = np.asarray(ln_beta, dtype=np.float32)
    w_qkv = np.asarray(w_qkv, dtype=np.float32)
    b_qkv = np.asarray(b_qkv, dtype=np.float32)
    w_proj = np.asarray(w_proj, dtype=np.float32)

    W = ln_gamma[:, None] * w_qkv          # fold gamma
    beff = b_qkv + ln_beta @ w_qkv         # fold beta
    ident = np.eye(128, dtype=np.float32).astype(BF16)

    in_maps = []
    for c in range(8):
        b, half = divmod(c, 2)
        hs = half * INNER_L
        wq = W[:, hs:hs + INNER_L] * SCALE
        wk = W[:, D + hs:D + hs + INNER_L]
        wv = W[:, 2 * D + hs:2 * D + hs + INNER_L]
        bq = beff[hs:hs + INNER_L] * SCALE
        bk = beff[D + hs:D + hs + INNER_L]
        bv = beff[2 * D + hs:2 * D + hs + INNER_L]
        wqkv_c = np.ascontiguousarray(
            np.concatenate([wq, wk, wv], axis=1)
        ).astype(BF16)
        bqk_col = np.ascontiguousarray(
            np.concatenate([bq, bk]).reshape(8, 128).T
        )
        bv_bc = np.ascontiguousarray(np.broadcast_to(bv[None, :], (128, INNER_L)))
        wproj_c = np.ascontiguousarray(w_proj[hs:hs + INNER_L, :]).astype(BF16)
        in_maps.append({
            "x": np.ascontiguousarray(x[b]).astype(BF16),
            "wqkv": wqkv_c,
            "bqk": bqk_col,
            "bv": bv_bc,
            "wproj": wproj_c,
            "ident": ident,
        })
    return in_maps


def kernel(x, ln_gamma, ln_beta, w_qkv, b_qkv, w_proj, b_proj, _trace=False, _tmpdir=None):
    from concourse.bass_utils import run_bass_kernel_spmd

    if "nc" not in _CACHE:
        _CACHE["nc"] = _build_nc()
    nc = _CACHE["nc"]

    in_maps = _prep_in_maps(x, ln_gamma, ln_beta, w_qkv, b_qkv, w_proj)
    res = run_bass_kernel_spmd(
        nc, in_maps, core_ids=list(range(8)), trace=_trace, tmpdir=_tmpdir
    )
    _CACHE["last_result"] = res

    b_proj = np.asarray(b_proj, dtype=np.float32)
    out = np.empty((4, N, D), dtype=np.float32)
    for b in range(4):
        out[b] = (res.results[2 * b]["out"].astype(np.float32)
                  + res.results[2 * b + 1]["out"].astype(np.float32) + b_proj)
    return out
